# revision 3
# baseline (speedup 1.0000x reference)
"""Exact self-kNN (k=32) on 8 TRN2 NeuronCores — v2 (fp32r + packed top-k).

Per core (SPMD): 2048 query rows, full 16384-col database, streamed by
512-col chunks.

Score: S'[q,j] = t_q . t_j - sq_j/2 - c_q   (t = RNE-11-bit fp32r rounding of
x, done host-side; c_q = fp16(sq_q/2 - 200) recenters each row's candidate
band near 0 so an fp16 eviction keeps ~2^-11 relative precision there).
Computed per (chunk, qtile) as ONE PSUM accumulation group of 3 matmuls:
  fp16 bias mm  ([p0;p1;p2;1s] x [nsq parts; -c_q rows]) + 2 fp32r main mms.

Top-k: ACT evicts PSUM to fp16 into the HIGH half-words of a packed u32
buffer whose LOW half-words hold a pre-written complemented local column
index (W-1-l). Viewed as fp32, packed ordering == (value desc, column asc),
so a single max8 per 1024-col window returns the top-8 values WITH their
columns embedded — no find_index8, no gather. A u32 scalar add rebases the
field to 16383-global. Merge: 4 x (max8 + match_replace) over the 128
candidates. Decode: value = high half-word as fp16; idx = 16383 - low field.
d = gam_q - 2*value, diagonal forced to 0.
"""

import numpy as np

N = 16384
D = 256
K = 32
NCORES = 8
QPC = N // NCORES          # 2048
QTILES = QPC // 128        # 16
CHUNK = 512
NCH = N // CHUNK           # 32
W = 1024                   # selection window (must be multiple of CHUNK)
WPC = W // CHUNK           # chunks per window
NWIN = N // W              # 16
NCAND = NWIN * 8           # 128

_nc_cache = None


def _build():
    import concourse.bacc as bacc
    import concourse.mybir as mybir
    import concourse.tile as tile

    nc = bacc.Bacc(trn_type="TRN2")
    f32, f16, f32r = mybir.dt.float32, mybir.dt.float16, mybir.dt.float32r
    u16, u32, i32 = mybir.dt.uint16, mybir.dt.uint32, mybir.dt.int32

    t0T_in = nc.dram_tensor("t0T", [128, N], f32r, kind="ExternalInput")
    t1T_in = nc.dram_tensor("t1T", [128, N], f32r, kind="ExternalInput")
    tq0_in = nc.dram_tensor("tq0", [128, QPC], f32r, kind="ExternalInput")
    tq1_in = nc.dram_tensor("tq1", [128, QPC], f32r, kind="ExternalInput")
    nsq4_in = nc.dram_tensor("nsq4", [4, N], f16, kind="ExternalInput")
    bq4_in = nc.dram_tensor("bq4", [4, QPC], f16, kind="ExternalInput")
    gam_in = nc.dram_tensor("gam", [128, QTILES], f32, kind="ExternalInput")

    out_i = nc.dram_tensor("out_i", [QPC, K], i32, kind="ExternalOutput")
    out_d = nc.dram_tensor("out_d", [QPC, K], f32, kind="ExternalOutput")

    with tile.TileContext(nc) as tc:
        with (
            tc.tile_pool(name="db", bufs=1) as db,
            tc.tile_pool(name="ld", bufs=2) as ld,
            tc.tile_pool(name="work", bufs=2) as work,
            tc.tile_pool(name="ps", bufs=4, space="PSUM") as ps,
        ):
            tq0 = db.tile([128, QPC], f32r)
            nc.sync.dma_start(tq0[:], tq0_in[:])
            tq1 = db.tile([128, QPC], f32r)
            nc.sync.dma_start(tq1[:], tq1_in[:])
            nsq4 = db.tile([4, N], f16)
            nc.sync.dma_start(nsq4[:], nsq4_in[:])
            bq4 = db.tile([4, QPC], f16)
            nc.sync.dma_start(bq4[:], bq4_in[:])
            gam = db.tile([128, QTILES], f32)
            nc.sync.dma_start(gam[:], gam_in[:])

            # packed score buffers: one W-wide u32 window per query tile.
            # (f32-typed for max8; low u16 lanes = complemented local col.)
            sbuf = db.tile([128, 12 * W], f32)
            cand = db.tile([128, QTILES * NCAND], f32)

            # one-time: complemented local iota (W-1-l) into the even u16
            # lanes of each tile's window buffer.
            iota_d = db.tile([128, W], u16)
            nc.gpsimd.iota(iota_d[:], pattern=[[-1, W]], base=W - 1,
                           channel_multiplier=0)
            sb16 = sbuf[:].bitcast(f16).rearrange(
                "p (w two) -> p w two", two=2)
            sbu16 = sbuf[:].bitcast(u16).rearrange(
                "p (w two) -> p w two", two=2)
            for t in range(12):
                lo = sbu16[:, t * W:(t + 1) * W, 0:1].rearrange(
                    "p w one -> p (w one)")
                nc.vector.tensor_copy(lo, iota_d[:])

            # ------------- main loop: 4-chunk visits, tiles split in 2 groups
            # Per (visit, qtile): 12 matmuls in stationary-major order (3
            # stationary loads -- f32r LDW is not shadow-loaded, so changes
            # cost ~110ns each and are amortized over 4 chunks), four
            # accumulation groups into the quarters of one 4-bank [128, 2048]
            # PSUM tile, two W-wide strided ACT evicts, two max8.
            # Tile groups: merges of group 0 overlap group 1's compute.
            VC = 4                       # chunks per visit
            GROUPS = [range(0, 6), range(6, 12), range(12, 16)]
            GT = 6                       # max tiles per group (buffer count)
            for grp, tiles in enumerate(GROUPS):
                for v in range(NCH // VC):
                    cbase = VC * v
                    dmas = []
                    for j in range(VC):
                        csj = slice(CHUNK * (cbase + j), CHUNK * (cbase + j + 1))
                        d0 = ld.tile([128, CHUNK], f32r, tag=f"t0_{j}",
                                     name=f"d0_{grp}_{v}_{j}")
                        nc.sync.dma_start(d0[:], t0T_in[:, csj])
                        d1 = ld.tile([128, CHUNK], f32r, tag=f"t1_{j}",
                                     name=f"d1_{grp}_{v}_{j}")
                        nc.sync.dma_start(d1[:], t1T_in[:, csj])
                        dmas.append((csj, d0, d1))
                    for t in tiles:
                        qs = slice(128 * t, 128 * (t + 1))
                        tb = t - tiles[0]
                        pw0 = ps.tile([128, W], f32, tag="ps",
                                      name=f"pw0_{grp}_{v}_{t}")
                        pw1 = ps.tile([128, W], f32, tag="ps",
                                      name=f"pw1_{grp}_{v}_{t}")
                        pws = [pw0, pw0, pw1, pw1]
                        def _sl(j):
                            return pws[j][:, (j % WPC) * CHUNK:
                                          (j % WPC + 1) * CHUNK]
                        for j, (csj, d0, d1) in enumerate(dmas):
                            nc.tensor.matmul(_sl(j), bq4[:, qs], nsq4[:, csj],
                                             start=True, stop=False)
                        for j, (csj, d0, d1) in enumerate(dmas):
                            nc.tensor.matmul(_sl(j), tq0[:, qs], d0[:],
                                             start=False, stop=False)
                        for j, (csj, d0, d1) in enumerate(dmas):
                            nc.tensor.matmul(_sl(j), tq1[:, qs], d1[:],
                                             start=False, stop=True)
                        for h in range(VC // WPC):
                            w = (cbase + WPC * h) // WPC
                            tb2 = tb * 2 + h
                            hi = sb16[:, tb2 * W:(tb2 + 1) * W, 1:2].rearrange(
                                "p w one -> p (w one)")
                            nc.scalar.copy(hi, pws[2 * h][:])
                            cslot = cand[:, t * NCAND + 8 * w:
                                         t * NCAND + 8 * w + 8]
                            nc.vector.max(out=cslot,
                                          in_=sbuf[:, tb2 * W:(tb2 + 1) * W])
                            off = N - W * (w + 1)
                            if off:
                                # rebase the index field (low u16 lane) only —
                                # a u32-wide add would round the packed word
                                # in the DVE's internal fp32 path.
                                fldv = cslot.bitcast(u16).rearrange(
                                    "p (k two) -> p k two",
                                    two=2)[:, :, 0:1].rearrange(
                                    "p k one -> p (k one)")
                                nc.vector.tensor_scalar_add(fldv, fldv,
                                                            float(off))
                # merge + decode for this group (overlaps next group compute)
                for t in tiles:
                    qs = slice(128 * t, 128 * (t + 1))
                    vw = cand[:, t * NCAND:(t + 1) * NCAND]
                    v32 = work.tile([128, K], f32, tag="v32")
                    for r in range(4):
                        nc.vector.max(out=v32[:, 8 * r:8 * r + 8], in_=vw)
                        if r < 3:
                            nc.vector.match_replace(
                                out=vw, in_to_replace=v32[:, 8 * r:8 * r + 8],
                                in_values=vw, imm_value=-3e38)
                    # decode: value = high fp16 lane; idx = 16383 - low field
                    v16 = v32[:].bitcast(f16).rearrange(
                        "p (k two) -> p k two", two=2)[:, :, 1:2].rearrange(
                        "p k one -> p (k one)")
                    fld = v32[:].bitcast(u16).rearrange(
                        "p (k two) -> p k two", two=2)[:, :, 0:1].rearrange(
                        "p k one -> p (k one)")
                    idxu = work.tile([128, K], u32, tag="idxu")
                    nc.vector.tensor_scalar(
                        out=idxu[:], in0=fld, scalar1=-1.0,
                        scalar2=float(N - 1),
                        op0=mybir.AluOpType.mult, op1=mybir.AluOpType.add)
                    d32 = work.tile([128, K], f32, tag="d32")
                    nc.vector.scalar_tensor_tensor(
                        out=d32[:], in0=v16, scalar=-2.0,
                        in1=gam[:, t:t + 1].to_broadcast([128, K]),
                        op0=mybir.AluOpType.mult, op1=mybir.AluOpType.add)
                    nc.vector.memset(d32[:, 0:1], 0.0)
                    nc.sync.dma_start(out_i[qs, :], idxu[:].bitcast(i32))
                    nc.sync.dma_start(out_d[qs, :], d32[:])

    nc.finalize()
    return nc


def _round11(a):
    """RNE to 11 stored mantissa bits (matches TRN2 fp32r operand rounding)."""
    m, e = np.frexp(a.astype(np.float64))
    s = np.ldexp(1.0, 12)
    mq = np.rint(m * s) / s
    return np.ldexp(mq, e).astype(np.float32)


def _prep(x):
    x = np.ascontiguousarray(np.asarray(x, dtype=np.float32))
    t = _round11(x)
    tT = np.ascontiguousarray(t.T)                      # [256, N]
    sq = (x.astype(np.float64) ** 2).sum(1).astype(np.float32)
    c = (sq / 2 - 200.0).astype(np.float16).astype(np.float32)
    nsq = -(sq.astype(np.float64) / 2)
    p0 = nsq.astype(np.float16)
    p1 = (nsq - p0.astype(np.float64)).astype(np.float16)
    p2 = (nsq - p0.astype(np.float64) - p1.astype(np.float64)).astype(
        np.float16)
    nsq4 = np.stack([p0, p1, p2, np.ones(N, np.float16)])  # [4, N]
    gam_full = sq - 2 * c                                  # [N]
    return tT, c, nsq4, gam_full


def kernel(x, k):
    from concourse.bass_utils import run_bass_kernel_spmd

    global _nc_cache
    assert int(k) == K
    tT, c, nsq4, gam_full = _prep(x)

    if _nc_cache is None:
        _nc_cache = _build()
    nc = _nc_cache

    in_maps = []
    for ci in range(NCORES):
        qs = slice(ci * QPC, (ci + 1) * QPC)
        bq4 = np.ones((4, QPC), np.float16)
        bq4[3] = (-c[qs]).astype(np.float16)
        gam = np.ascontiguousarray(
            gam_full[qs].reshape(QTILES, 128).T.astype(np.float32))
        in_maps.append({
            "t0T": tT[:128], "t1T": tT[128:],
            "tq0": np.ascontiguousarray(tT[:128, qs]),
            "tq1": np.ascontiguousarray(tT[128:, qs]),
            "nsq4": nsq4, "bq4": bq4, "gam": gam,
        })
    res = run_bass_kernel_spmd(nc, in_maps, core_ids=list(range(NCORES)))
    idx = np.concatenate([r["out_i"] for r in res.results], axis=0).astype(np.int32)
    dist = np.concatenate([r["out_d"] for r in res.results], axis=0).astype(np.float32)
    return idx, dist
